# Initial kernel scaffold
#
"""Trainium2 Bass kernel for a 3-layer LSTM (B=4096, T=1024, IN=2, H=16) + final FC.

Per core (batch-sharded 8 ways, B_local=512), wavefront over layers:
macro-step s computes L0@t=s, L1@t=s-1, L2@t=s-2. All weights resident in the
PE array (MM1: rows 0:64 x cols 0:128 for L0+L1; MM2: rows 64:113 x cols 0:128
for L2, zero-padded). Biases enter via constant-1 rows of the state.

Partition-alignment rules (hardware): every compute-engine operand must start
at partition 0/32/64/96, and tensor_tensor inputs must share the same start.
Layout is built around that:
  PSUM P [128,1024] f32: free 0:512 = MM1 out, 512:1024 = MM2 out.
    rows: 0:32 = [i0;i1], 32:64 = [f0;f1], 64:96 = [o0;o1], 96:128 = [g0;g1]
    (MM2/free2: same bases, L2 group in lower 16 rows of each quadrant).
  S [128,512] fp16 (matmul rhs): 16:18 x_t (DMA'd), 18 one, 32:48 h0,
    48:64 h1, 80:96 h1' (upper half of a duplicated H-op), 96:112 h2, 112 one.
  tanh-gates pre-scaled x2 in weights; tanh(x)=2*sigmoid(2x)-1 fixup on ACT.
"""

import os
import sys

sys.path.insert(0, "/opt/trn_rl_repo")

import numpy as np

import concourse.bacc as bacc
import concourse.mybir as mybir
from concourse.tile import TileContext
from concourse import bass_utils

B, T, IN, H, L = 4096, 1024, 2, 16, 3
NCORES = 8
BL = B // NCORES  # 512
F32 = mybir.dt.float32
F16 = mybir.dt.float16
NPF16 = np.float16

# PyTorch gate rows in W_ih*/W_hh*: i, f, g, o
PT_I, PT_F, PT_G, PT_O = slice(0, 16), slice(16, 32), slice(32, 48), slice(48, 64)

_STEPS_ENV = int(os.environ.get("LSTM_STEPS", "0"))
_CDT_ENV = os.environ.get("LSTM_CDT", "f32")  # cell-state dtype: f32 | f16


def _t_run():
    return _STEPS_ENV if _STEPS_ENV > 0 else T


def build_weight_block(W_ih0, W_hh0, b0, W_ih1, W_hh1, b1, W_ih2, W_hh2, b2):
    """WT [128,128] f32.

    MM1 = WT[0:64, 0:128]: K rows: 16:18 x, 18 one, 32:48 h0, 48:64 h1.
      cols (psum bank0): 0:16 i0, 16:32 i1, 32:48 f0, 48:64 f1,
                         64:80 o0, 80:96 o1, 96:112 2*g0, 112:128 2*g1.
    MM2 = WT[64:113, 0:128]: K rows: 80:96 h1', 96:112 h2, 112 one.
      cols (psum bank1): 0:16 i2, 32:48 f2, 64:80 o2, 96:112 2*g2, rest 0.
    """
    WT = np.zeros((128, 128), np.float32)
    # layer0: (pt_gate, col, scale); rec rows 32:48, in rows 16:18, bias 18
    for pt, c0, sc in ((PT_I, 0, 1.0), (PT_F, 32, 1.0), (PT_O, 64, 1.0), (PT_G, 96, 2.0)):
        WT[32:48, c0:c0 + 16] = sc * W_hh0[pt].T
        WT[16:18, c0:c0 + 16] = sc * W_ih0[pt].T
        WT[18, c0:c0 + 16] = sc * b0[pt]
    # layer1: in rows 32:48 (h0), rec rows 48:64 (h1), bias 18
    for pt, c0, sc in ((PT_I, 16, 1.0), (PT_F, 48, 1.0), (PT_O, 80, 1.0), (PT_G, 112, 2.0)):
        WT[32:48, c0:c0 + 16] = sc * W_ih1[pt].T
        WT[48:64, c0:c0 + 16] = sc * W_hh1[pt].T
        WT[18, c0:c0 + 16] = sc * b1[pt]
    # layer2: in rows 80:96 (h1'), rec rows 96:112 (h2), bias 112
    for pt, c0, sc in ((PT_I, 0, 1.0), (PT_F, 32, 1.0), (PT_O, 64, 1.0), (PT_G, 96, 2.0)):
        WT[80:96, c0:c0 + 16] = sc * W_ih2[pt].T
        WT[96:112, c0:c0 + 16] = sc * W_hh2[pt].T
        WT[112, c0:c0 + 16] = sc * b2[pt]
    return WT


def build_bass():
    CDT = F32 if _CDT_ENV == "f32" else F16
    NPC = np.float32 if _CDT_ENV == "f32" else NPF16
    nc = bacc.Bacc("TRN2", target_bir_lowering=False, debug=False,
                   num_devices=NCORES)
    NT = _t_run() + 2
    nblk = (_t_run() + 63) // 64

    xt_d = nc.dram_tensor("xt", [128, nblk * BL], F16, kind="ExternalInput")
    wt_d = nc.dram_tensor("wt", [128, 128], F16, kind="ExternalInput")
    wfc_d = nc.dram_tensor("wfc", [17, 1], F16, kind="ExternalInput")
    s0_d = nc.dram_tensor("s0", [128, BL], F16, kind="ExternalInput")
    c0_d = nc.dram_tensor("c0", [32, 2 * BL], mybir.dt.from_np(np.dtype(NPC)),
                          kind="ExternalInput")
    h1i_d = nc.dram_tensor("h1i", [16, BL], F16, kind="ExternalInput")
    c1i_d = nc.dram_tensor("c1i", [16, BL], mybir.dt.from_np(np.dtype(NPC)),
                           kind="ExternalInput")
    h2i_d = nc.dram_tensor("h2i", [16, BL], F16, kind="ExternalInput")
    c2i_d = nc.dram_tensor("c2i", [16, BL], mybir.dt.from_np(np.dtype(NPC)),
                           kind="ExternalInput")
    y_d = nc.dram_tensor("y", [1, BL], F32, kind="ExternalOutput")

    SIG = mybir.ActivationFunctionType.Sigmoid
    TANH = mybir.ActivationFunctionType.Tanh
    CPY = mybir.ActivationFunctionType.Copy

    with TileContext(nc) as tc:
        xt = nc.alloc_sbuf_tensor("xt_sb", [128, nblk * BL], F16)
        wt = nc.alloc_sbuf_tensor("wt_sb", [128, 128], F16)
        wfc = nc.alloc_sbuf_tensor("wfc_sb", [17, 1], F16)
        S = nc.alloc_sbuf_tensor("S_sb", [128, BL], F16)
        C = nc.alloc_sbuf_tensor("C_sb", [64, 2 * BL], CDT)
        hf = nc.alloc_sbuf_tensor("hf_sb", [17, BL], F16)
        ys = nc.alloc_sbuf_tensor("ys_sb", [1, BL], F32)

        nc.sync.dma_start(xt[:, :], xt_d.ap())
        nc.sync.dma_start(wt[:, :], wt_d.ap())
        nc.sync.dma_start(wfc[:, :], wfc_d.ap())
        nc.sync.dma_start(S[:, :], s0_d.ap())
        nc.sync.dma_start(C[32:64, :], c0_d.ap())

        with tc.tile_pool(name="ps", bufs=2, space="PSUM") as pps, \
             tc.tile_pool(name="sb", bufs=3) as psb:
            for s in range(NT):
                P = pps.tile([128, 1024], F32, tag="P")
                G = psb.tile([128, 1024], F16, tag="G")
                TG = psb.tile([32, 1024], F16, tag="TG")
                U = psb.tile([64, 1024], F16, tag="U")
                FW = psb.tile([64, 1024], CDT, tag="FW")
                TC = psb.tile([96, 1024], F16, tag="TC")

                nc.tensor.matmul(P[0:128, 0:512], wt[0:64, 0:128], S[0:64, :],
                                 start=True, stop=True)
                nc.tensor.matmul(P[0:128, 512:1024], wt[64:113, 0:128],
                                 S[64:113, :], start=True, stop=True)
                # all gates in one sigmoid (g pre-scaled x2 in weights)
                nc.scalar.activation(G[0:128, 0:1024], P[0:128, 0:1024], SIG)
                # tanh(g) = 2*sigmoid(2g)-1  (on ACT to offload DVE)
                nc.scalar.activation(TG[0:32, 0:1024], G[96:128, 0:1024], CPY,
                                     bias=-1.0, scale=2.0)
                # u = i * tanh(g)
                nc.vector.tensor_mul(U[32:64, 0:1024], G[0:32, 0:1024],
                                     TG[0:32, 0:1024])
                # fw = f * c
                nc.vector.tensor_mul(FW[32:64, 0:1024], G[32:64, 0:1024],
                                     C[32:64, 0:1024])
                # c' = fw + u
                nc.vector.tensor_add(C[32:64, 0:1024], FW[32:64, 0:1024],
                                     U[32:64, 0:1024])
                # tc = tanh(c')
                nc.scalar.activation(TC[64:96, 0:1024], C[32:64, 0:1024], TANH)
                # h = o * tc: [h0;h1] -> S[32:64]; dup -> S[64:96] (h1' at 80:96)
                nc.vector.tensor_mul(S[32:64, :], G[64:96, 0:512],
                                     TC[64:96, 0:512])
                nc.vector.tensor_mul(S[64:96, :], G[64:96, 0:512],
                                     TC[64:96, 0:512])
                # h2 -> S[96:112]
                nc.vector.tensor_mul(S[96:112, :], G[64:80, 512:1024],
                                     TC[64:80, 512:1024])
                # stage next x
                if s + 1 < _t_run():
                    nb_, nu = divmod(s + 1, 64)
                    nc.sync.dma_start(
                        S[16:18, :], xt[2 * nu:2 * nu + 2, nb_ * BL:(nb_ + 1) * BL])
                # delayed init: overwrite wavefront-startup pollution
                if s == 0:
                    nc.sync.dma_start(S[48:64, :], h1i_d.ap())
                    nc.sync.dma_start(C[48:64, 0:512], c1i_d.ap())
                elif s == 1:
                    nc.sync.dma_start(S[96:112, :], h2i_d.ap())
                    nc.sync.dma_start(C[32:48, 512:1024], c2i_d.ap())

            # final fc: y = h2 @ W_fc.T + b_fc
            nc.vector.memset(hf[0:17, :], 1.0)
            nc.vector.tensor_copy(hf[0:16, :], S[96:112, :])
            PF = pps.tile([1, BL], F32, tag="PF")
            nc.tensor.matmul(PF[0:1, :], wfc[0:17, 0:1], hf[0:17, :],
                             start=True, stop=True)
            nc.scalar.copy(ys[0:1, :], PF[0:1, :])
            nc.sync.dma_start(y_d.ap(), ys[0:1, :])

    nc.compile()
    return nc


def prep_core_inputs(inputs, core):
    NPC = np.float32 if _CDT_ENV == "f32" else NPF16
    b0, b1 = core * BL, (core + 1) * BL
    tr = _t_run()
    nblk = (tr + 63) // 64

    x = np.asarray(inputs["x"])[b0:b1]          # [BL, T, IN]
    h0 = np.asarray(inputs["h0"])[:, b0:b1]     # [L, BL, H]
    c0 = np.asarray(inputs["c0"])[:, b0:b1]

    # xt layout: partition = 2*(t%64)+f, free = (t//64)*BL + b
    xt = np.zeros((128, nblk * BL), np.float32)
    xr = x[:, :tr, :].transpose(1, 2, 0)         # [t, f, b]
    for tb in range(nblk):
        t1 = min(tb * 64 + 64, tr)
        chunk = xr[tb * 64:t1]                   # [u, f, b]
        xt[:2 * (t1 - tb * 64), tb * BL:(tb + 1) * BL] = chunk.reshape(-1, BL)

    s0 = np.zeros((128, BL), np.float32)
    s0[16:18] = x[:, 0, :].T
    s0[18] = 1.0
    s0[32:48] = h0[0].T
    s0[48:64] = h0[1].T
    s0[80:96] = h0[1].T
    s0[96:112] = h0[2].T
    s0[112] = 1.0

    c0p = np.zeros((32, 2 * BL), np.float32)
    c0p[0:16, 0:BL] = c0[0].T
    c0p[16:32, 0:BL] = c0[1].T
    c0p[0:16, BL:2 * BL] = c0[2].T

    return {
        "xt": xt.astype(NPF16),
        "s0": s0.astype(NPF16),
        "c0": c0p.astype(NPC),
        "h1i": np.ascontiguousarray(h0[1].T).astype(NPF16),
        "c1i": np.ascontiguousarray(c0[1].T).astype(NPC),
        "h2i": np.ascontiguousarray(h0[2].T).astype(NPF16),
        "c2i": np.ascontiguousarray(c0[2].T).astype(NPC),
    }


_NC_CACHE = {}


def kernel(**inputs):
    key = (_t_run(), _CDT_ENV)
    if key not in _NC_CACHE:
        _NC_CACHE[key] = build_bass()
    nc = _NC_CACHE[key]

    b0v = np.asarray(inputs["b_ih0"]) + np.asarray(inputs["b_hh0"])
    b1v = np.asarray(inputs["b_ih1"]) + np.asarray(inputs["b_hh1"])
    b2v = np.asarray(inputs["b_ih2"]) + np.asarray(inputs["b_hh2"])
    WT = build_weight_block(
        np.asarray(inputs["W_ih0"]), np.asarray(inputs["W_hh0"]), b0v,
        np.asarray(inputs["W_ih1"]), np.asarray(inputs["W_hh1"]), b1v,
        np.asarray(inputs["W_ih2"]), np.asarray(inputs["W_hh2"]), b2v,
    ).astype(NPF16)
    wfc = np.zeros((17, 1), np.float32)
    wfc[0:16, 0] = np.asarray(inputs["W_fc"])[0]
    wfc[16, 0] = np.asarray(inputs["b_fc"])[0]
    wfc = wfc.astype(NPF16)

    in_maps = []
    for core in range(NCORES):
        m = prep_core_inputs(inputs, core)
        m["wt"] = WT
        m["wfc"] = wfc
        in_maps.append(m)

    trace = os.environ.get("LSTM_TRACE", "0") == "1"
    res = bass_utils.run_bass_kernel_spmd(nc, in_maps, core_ids=list(range(NCORES)),
                                          trace=trace)
    global _LAST_RESULT
    _LAST_RESULT = res
    out = np.concatenate([res.results[c]["y"][0] for c in range(NCORES)])
    return out.reshape(B, 1).astype(np.float32)


_LAST_RESULT = None


if __name__ == "__main__":
    import reference
    inputs = reference.setup_inputs()
    y = kernel(**{k: np.asarray(v) for k, v in inputs.items()})
    print("kernel out", y.shape, y[:4, 0])



# revision 24
# speedup vs baseline: 1.9213x; 1.9213x over previous
"""Trainium2 Bass kernel for a 3-layer LSTM (B=4096, T=1024, IN=2, H=16) + final FC.

Per core (batch-sharded 8 ways, B_local=512), wavefront over layers: macro-step
s computes L0@t=s, L1@t=s-1, L2@t=s-2. Two independent batch streams (256 each)
interleave to hide the recurrence-cycle latency.

Gate layout (per stream): two PSUM tiles [128, 256] f32 in SEPARATE banks.
  Pi {i|f} gates:  partitions 0:64 = i-features, 64:128 = f-features
  Pg {g|o} gates:  partitions 0:64 = 2*g-features, 64:128 = o-features
  feature order within 64: L0 0:16, L1 16:32, L2 32:48, pad 48:64.
  (G in SBUF keeps the merged [128, 512] layout: if at 0:256, go at 256:512.)
Every pointwise pairing (i vs g, f vs c, o vs tanh(c)) is then either on the
same partitions at different free offsets or on {0:64}/{64:128} aligned starts,
so the cell update is 5 element-wise ops + 1 sigmoid + 1 tanh per stream:
  G = sigmoid(P)            (g cols pre-scaled x2 in weights)
  tg = 2*G_g - 1            (tanh fixup, DVE tensor_scalar)
  u  = G_i * tg             c' = f*c + u        tc = tanh(c')
  h  = G_o * tc  -> S[64:128]  (all three layers' h in ONE op)

x(t) never touches the S state (that would put a DMA on the recurrence cycle):
its gate contribution comes from an accumulating matmul per PSUM half,
streaming xt32 [32, T/16*BL] (partition = 2*(t%16)+f, aligned start 0) against
one of 16 per-(t%16) stationary variants whose other 30 rows are zero. These
x matmuls are PREFILLED (start=True) into the next step's tiles at the end of
each stream block: accumulation groups may interleave across banks (verified),
only never within one bank, so the h-dependent matmuls (start=False, stop=True)
are the only PE work on the recurrence cycle.

S state [128, 256] per stream (only rows 64:128 streamed, K=64): 64:80 h0,
80:96 h1, 96:112 h2, 112 bias-one (the pad feature engineered to stay exactly
1.0 through the h write: i-bias -20, f/o-bias +20, cell init 10), 113:128
zero-weighted garbage. Engine ops may only span 64 partitions when starting at
0 or 64, which fixes the whole row layout. PSUM accumulation groups must stay
consecutive per region (interleaved start/stop groups corrupt results on HW).
"""

import os
import sys

sys.path.insert(0, "/opt/trn_rl_repo")

import numpy as np

import concourse.bacc as bacc
import concourse.mybir as mybir
from concourse.tile import TileContext
from concourse import bass_utils

B, T, IN, H, L = 4096, 1024, 2, 16, 3
NCORES = 8
BL = B // NCORES        # 512 per core
SB = BL // 2            # 256 per stream
F32 = mybir.dt.float32
F16 = mybir.dt.float16
NPF16 = np.float16

# PyTorch gate rows in W_ih*/W_hh*: i, f, g, o
PT = {"i": slice(0, 16), "f": slice(16, 32), "g": slice(32, 48), "o": slice(48, 64)}

_STEPS_ENV = int(os.environ.get("LSTM_STEPS", "0"))
_CDT_ENV = os.environ.get("LSTM_CDT", "f16")  # cell-state dtype: f32 | f16
_POOL_H_ENV = os.environ.get("LSTM_POOL_H", "0") == "1"  # h-mul on gpsimd


def _t_run():
    return _STEPS_ENV if _STEPS_ENV > 0 else T


def build_weight_block(Ws):
    """WT [96, 256] f32: cols 0:128 {i|f}, 128:256 {g|o}; within a block cols
    0:64 first gate, 64:128 second; feature = 16*l + j.
    Rows (only 64:128 streamed, K=64): 64:80 h0, 80:96 h1, 96:112 h2,
    112 bias. Row 112 of S is the pad feature 48, engineered to stay exactly
    1.0: its i-gate bias is -20 (i=0), f/o-gate biases +20 (f=o=1), and its
    cell is initialized to 10, so h_pad = 1*tanh(10) == 1.0 in fp16."""
    WT = np.zeros((128, 256), np.float32)
    for bank, gates in ((0, ("i", "f")), (1, ("g", "o"))):
        for gi, gate in enumerate(gates):
            sc = 2.0 if gate == "g" else 1.0
            pt = PT[gate]
            for l in range(3):
                Wih, Whh, b = Ws[l]
                c0 = bank * 128 + 64 * gi + 16 * l
                if l > 0:
                    WT[64 + 16 * (l - 1):80 + 16 * (l - 1), c0:c0 + 16] = \
                        sc * Wih[pt].T
                WT[64 + 16 * l:80 + 16 * l, c0:c0 + 16] = sc * Whh[pt].T
                WT[112, c0:c0 + 16] = sc * b[pt]
            pc = bank * 128 + 64 * gi + 48      # pad feature col
            WT[112, pc] = {"i": -20.0, "f": 20.0, "o": 20.0, "g": 0.0}[gate]
    return WT


def build_x_weights(W_ih0):
    """wx [32, 16*256] f32: variant v=t%16 at cols v*256; within: 0:128
    {i|f}-block, 128:256 {g|o}-block; only rows 2v:2v+2 nonzero (L0 features,
    cols +0:16 of each 64-gate half)."""
    wx = np.zeros((32, 16 * 256), np.float32)
    for v in range(16):
        for bank, gates in ((0, ("i", "f")), (1, ("g", "o"))):
            for gi, gate in enumerate(gates):
                sc = 2.0 if gate == "g" else 1.0
                c0 = v * 256 + bank * 128 + 64 * gi
                wx[2 * v:2 * v + 2, c0:c0 + 16] = sc * W_ih0[PT[gate]].T
    return wx


def build_bass():
    CDT = F32 if _CDT_ENV == "f32" else F16
    NPC = np.float32 if _CDT_ENV == "f32" else NPF16
    nc = bacc.Bacc("TRN2", target_bir_lowering=False, debug=False,
                   num_devices=NCORES)
    tr = _t_run()
    NT = tr + 2
    nblk16 = (tr + 15) // 16

    xt_d = nc.dram_tensor("xt", [32, nblk16 * BL], F16, kind="ExternalInput")
    wt_d = nc.dram_tensor("wt", [128, 256], F16, kind="ExternalInput")
    wx_d = nc.dram_tensor("wx", [32, 16 * 256], F16, kind="ExternalInput")
    wfc_d = nc.dram_tensor("wfc", [17, 1], F16, kind="ExternalInput")
    s0_d = [nc.dram_tensor(f"s0{c}", [128, SB], F16, kind="ExternalInput")
            for c in "ab"]
    c0_d = [nc.dram_tensor(f"c0{c}", [64, SB], mybir.dt.from_np(np.dtype(NPC)),
                           kind="ExternalInput") for c in "ab"]
    # delayed re-inits (overwrite wavefront startup pollution)
    ri_d = {}
    for c in "ab":
        for nm in ("h1i", "h2i"):
            ri_d[nm + c] = nc.dram_tensor(nm + c, [16, SB], F16,
                                          kind="ExternalInput")
        for nm in ("c1i", "c2i"):
            ri_d[nm + c] = nc.dram_tensor(
                nm + c, [16, SB], mybir.dt.from_np(np.dtype(NPC)),
                kind="ExternalInput")
    y_d = nc.dram_tensor("y", [1, BL], F32, kind="ExternalOutput")

    SIG = mybir.ActivationFunctionType.Sigmoid
    TANH = mybir.ActivationFunctionType.Tanh

    with TileContext(nc) as tc:
        xt = nc.alloc_sbuf_tensor("xt_sb", [32, nblk16 * BL], F16)
        wt = nc.alloc_sbuf_tensor("wt_sb", [128, 256], F16)
        wx = nc.alloc_sbuf_tensor("wx_sb", [32, 16 * 256], F16)
        wfc = nc.alloc_sbuf_tensor("wfc_sb", [17, 1], F16)
        St = [nc.alloc_sbuf_tensor(f"St{c}", [128, SB], F16) for c in "ab"]
        C = [nc.alloc_sbuf_tensor(f"C{c}", [128, SB], CDT) for c in "ab"]
        hf = nc.alloc_sbuf_tensor("hf_sb", [17, BL], F16)
        ys = nc.alloc_sbuf_tensor("ys_sb", [1, BL], F32)

        nc.sync.dma_start(xt[:, :], xt_d.ap())
        nc.sync.dma_start(wt[:, :], wt_d.ap())
        nc.sync.dma_start(wx[:, :], wx_d.ap())
        nc.sync.dma_start(wfc[:, :], wfc_d.ap())
        for k in range(2):
            nc.sync.dma_start(St[k][:, :], s0_d[k].ap())
            nc.sync.dma_start(C[k][64:128, :], c0_d[k].ap())

        with tc.tile_pool(name="psGoA", bufs=2, space="PSUM") as pgA, \
             tc.tile_pool(name="psGoB", bufs=2, space="PSUM") as pgB, \
             tc.tile_pool(name="psIfA", bufs=2, space="PSUM") as piA, \
             tc.tile_pool(name="psIfB", bufs=2, space="PSUM") as piB, \
             tc.tile_pool(name="sbA", bufs=4) as psbA, \
             tc.tile_pool(name="sbB", bufs=4) as psbB:
            pgo = [pgA, pgB]
            pif = [piA, piB]
            psb = [psbA, psbB]

            # x gate contributions are prefilled into the NEXT step's PSUM
            # tiles (start=True) at the end of each stream block, so only the
            # h-dependent matmuls (start=False) sit on the recurrence cycle.
            # The go/if halves live in SEPARATE pools (separate banks): within
            # any one bank, accumulation groups stay strictly sequential --
            # interleaving two open groups in one bank corrupts results.
            # emit go,go,if,if so same-stationary pairs are adjacent on
            # the PE queue (lets the compiler skip redundant weight loads)
            def prefill_x(s):
                v = (s % 16) * 256
                xf0 = (s // 16) * BL
                PgA = pgo[0].tile([128, 256], F32, tag="Pg")
                PgB = pgo[1].tile([128, 256], F32, tag="Pg")
                PiA = pif[0].tile([128, 256], F32, tag="Pi")
                PiB = pif[1].tile([128, 256], F32, tag="Pi")
                xmA = xt[0:32, xf0:xf0 + SB]
                xmB = xt[0:32, xf0 + SB:xf0 + BL]
                nc.tensor.matmul(PgA[0:128, 0:256],
                                 wx[0:32, v + 128:v + 256], xmA,
                                 start=True, stop=False)
                nc.tensor.matmul(PgB[0:128, 0:256],
                                 wx[0:32, v + 128:v + 256], xmB,
                                 start=True, stop=False)
                nc.tensor.matmul(PiA[0:128, 0:256], wx[0:32, v:v + 128], xmA,
                                 start=True, stop=False)
                nc.tensor.matmul(PiB[0:128, 0:256], wx[0:32, v:v + 128], xmB,
                                 start=True, stop=False)
                return [(PgA, PiA), (PgB, PiB)]

            Ptil = prefill_x(0)
            for s in range(NT):
                have_x = s < tr
                for k in range(2):
                    if have_x:
                        Pg, Pi = Ptil[k]
                    else:
                        Pg = pgo[k].tile([128, 256], F32, tag="Pg")
                        Pi = pif[k].tile([128, 256], F32, tag="Pi")
                    G = psb[k].tile([128, 512], F16, tag="G")
                    TG = psb[k].tile([64, 256], F16, tag="TG")
                    U = psb[k].tile([64, 256], F16, tag="U")
                    FW = psb[k].tile([64, 256], CDT, tag="FW")
                    TC = psb[k].tile([128, 256], F16, tag="TC")

                    nc.tensor.matmul(Pg[0:128, 0:256], wt[64:128, 128:256],
                                     St[k][64:128, :], start=not have_x,
                                     stop=True)
                    nc.tensor.matmul(Pi[0:128, 0:256], wt[64:128, 0:128],
                                     St[k][64:128, :], start=not have_x,
                                     stop=True)
                    # sigmoid per half ({g|o} first: it unblocks the DVE chain)
                    nc.scalar.activation(G[0:128, 256:512], Pg[0:128, 0:256],
                                         SIG)
                    nc.scalar.activation(G[0:128, 0:256], Pi[0:128, 0:256],
                                         SIG)
                    # tg = tanh(g) = 2*sigmoid(2g) - 1
                    nc.vector.tensor_scalar(
                        TG[0:64, 0:256], G[0:64, 256:512], 2.0, -1.0,
                        mybir.AluOpType.mult, mybir.AluOpType.add)
                    # fw = f * c
                    nc.vector.tensor_mul(FW[0:64, 0:256], G[64:128, 0:256],
                                         C[k][64:128, :])
                    # u = i * tg
                    nc.vector.tensor_mul(U[0:64, 0:256], TG[0:64, 0:256],
                                         G[0:64, 0:256])
                    # c' = fw + u
                    nc.vector.tensor_add(C[k][64:128, :], FW[0:64, 0:256],
                                         U[0:64, 0:256])
                    # tc = tanh(c')
                    nc.scalar.activation(TC[64:128, 0:256], C[k][64:128, :],
                                         TANH)
                    # h = o * tc -> all three layers' h rows at once
                    eng = nc.gpsimd if _POOL_H_ENV else nc.vector
                    eng.tensor_mul(St[k][64:128, :], G[64:128, 256:512],
                                   TC[64:128, 0:256])
                if s + 1 < tr:
                    Ptil = prefill_x(s + 1)

                # delayed init: overwrite wavefront-startup pollution
                if s == 0:
                    for k, c in enumerate("ab"):
                        nc.sync.dma_start(St[k][80:96, :], ri_d["h1i" + c].ap())
                        nc.sync.dma_start(C[k][80:96, :], ri_d["c1i" + c].ap())
                elif s == 1:
                    for k, c in enumerate("ab"):
                        nc.sync.dma_start(St[k][96:112, :], ri_d["h2i" + c].ap())
                        nc.sync.dma_start(C[k][96:112, :], ri_d["c2i" + c].ap())

            # final fc: y = h2 @ W_fc.T + b_fc
            nc.vector.memset(hf[0:17, :], 1.0)
            for k in range(2):
                nc.vector.tensor_copy(hf[0:16, k * SB:(k + 1) * SB],
                                      St[k][96:112, :])
            for k in range(2):
                Pf = pgo[k].tile([128, 256], F32, tag="Pg")
                nc.tensor.matmul(Pf[0:1, 0:256], wfc[0:17, 0:1],
                                 hf[0:17, k * SB:(k + 1) * SB],
                                 start=True, stop=True)
                nc.scalar.copy(ys[0:1, k * SB:(k + 1) * SB], Pf[0:1, 0:256])
            nc.sync.dma_start(y_d.ap(), ys[0:1, :])

    nc.compile()
    return nc


def prep_core_inputs(inputs, core):
    NPC = np.float32 if _CDT_ENV == "f32" else NPF16
    b0, b1 = core * BL, (core + 1) * BL
    tr = _t_run()
    nblk16 = (tr + 15) // 16

    x = np.asarray(inputs["x"])[b0:b1]          # [BL, T, IN]
    h0 = np.asarray(inputs["h0"])[:, b0:b1]     # [L, BL, H]
    c0 = np.asarray(inputs["c0"])[:, b0:b1]

    # xt layout: partition = 2*(t%16)+f, free = (t//16)*BL + b
    xt = np.zeros((32, nblk16 * BL), np.float32)
    xr = x[:, :tr, :].transpose(1, 2, 0)         # [t, f, b]
    for tb in range(nblk16):
        t1 = min(tb * 16 + 16, tr)
        chunk = xr[tb * 16:t1]                   # [u, f, b]
        xt[:2 * (t1 - tb * 16), tb * BL:(tb + 1) * BL] = chunk.reshape(-1, BL)

    m = {"xt": xt.astype(NPF16)}
    for k, c in enumerate("ab"):
        sl = slice(k * SB, (k + 1) * SB)
        s0 = np.zeros((128, SB), np.float32)
        s0[112] = 1.0
        for l in range(3):
            s0[64 + 16 * l:80 + 16 * l] = h0[l, sl].T
        c0b = np.zeros((64, SB), np.float32)
        c0b[48] = 10.0
        for l in range(3):
            c0b[16 * l:16 * l + 16] = c0[l, sl].T
        m["s0" + c] = s0.astype(NPF16)
        m["c0" + c] = c0b.astype(NPC)
        m["h1i" + c] = np.ascontiguousarray(h0[1, sl].T).astype(NPF16)
        m["c1i" + c] = np.ascontiguousarray(c0[1, sl].T).astype(NPC)
        m["h2i" + c] = np.ascontiguousarray(h0[2, sl].T).astype(NPF16)
        m["c2i" + c] = np.ascontiguousarray(c0[2, sl].T).astype(NPC)
    return m


_NC_CACHE = {}


def kernel(**inputs):
    key = (_t_run(), _CDT_ENV, _POOL_H_ENV)
    if key not in _NC_CACHE:
        _NC_CACHE[key] = build_bass()
    nc = _NC_CACHE[key]

    Ws = []
    for l in range(3):
        Ws.append((np.asarray(inputs[f"W_ih{l}"]), np.asarray(inputs[f"W_hh{l}"]),
                   np.asarray(inputs[f"b_ih{l}"]) + np.asarray(inputs[f"b_hh{l}"])))
    WT = build_weight_block(Ws).astype(NPF16)
    WX = build_x_weights(Ws[0][0]).astype(NPF16)
    wfc = np.zeros((17, 1), np.float32)
    wfc[0:16, 0] = np.asarray(inputs["W_fc"])[0]
    wfc[16, 0] = np.asarray(inputs["b_fc"])[0]
    wfc = wfc.astype(NPF16)

    in_maps = []
    for core in range(NCORES):
        m = prep_core_inputs(inputs, core)
        m["wt"] = WT
        m["wx"] = WX
        m["wfc"] = wfc
        in_maps.append(m)

    trace = os.environ.get("LSTM_TRACE", "0") == "1"
    tmpdir = os.environ.get("LSTM_TMPDIR") or None
    res = bass_utils.run_bass_kernel_spmd(nc, in_maps, core_ids=list(range(NCORES)),
                                          trace=trace, tmpdir=tmpdir)
    global _LAST_RESULT
    _LAST_RESULT = res
    out = np.concatenate([res.results[c]["y"][0] for c in range(NCORES)])
    return out.reshape(B, 1).astype(np.float32)


_LAST_RESULT = None


if __name__ == "__main__":
    import reference
    inputs = reference.setup_inputs()
    y = kernel(**{k: np.asarray(v) for k, v in inputs.items()})
    print("kernel out", y.shape, y[:4, 0])


# revision 25
# speedup vs baseline: 1.9320x; 1.0055x over previous
"""Trainium2 Bass kernel for a 3-layer LSTM (B=4096, T=1024, IN=2, H=16) + final FC.

Per core (batch-sharded 8 ways, B_local=512), wavefront over layers: macro-step
s computes L0@t=s, L1@t=s-1, L2@t=s-2. Two independent batch streams (256 each)
interleave to hide the recurrence-cycle latency.

Gate layout (per stream): two PSUM tiles [128, 256] f32 in SEPARATE banks.
  Pi {i|f} gates:  partitions 0:64 = i-features, 64:128 = f-features
  Pg {g|o} gates:  partitions 0:64 = 2*g-features, 64:128 = o-features
  feature order within 64: L0 0:16, L1 16:32, L2 32:48, pad 48:64.
  (G in SBUF keeps the merged [128, 512] layout: if at 0:256, go at 256:512.)
Every pointwise pairing (i vs g, f vs c, o vs tanh(c)) is then either on the
same partitions at different free offsets or on {0:64}/{64:128} aligned starts,
so the cell update is 5 element-wise ops + 1 sigmoid + 1 tanh per stream:
  G = sigmoid(P)            (g cols pre-scaled x2 in weights)
  tg = 2*G_g - 1            (tanh fixup, DVE tensor_scalar)
  u  = G_i * tg             c' = f*c + u        tc = tanh(c')
  h  = G_o * tc  -> S[64:128]  (all three layers' h in ONE op)

x(t) never touches the S state (that would put a DMA on the recurrence cycle):
its gate contribution comes from an accumulating matmul per PSUM half,
streaming xt32 [32, T/16*BL] (partition = 2*(t%16)+f, aligned start 0) against
one of 16 per-(t%16) stationary variants whose other 30 rows are zero. These
x matmuls are PREFILLED (start=True) into the next step's tiles at the end of
each stream block: accumulation groups may interleave across banks (verified),
only never within one bank, so the h-dependent matmuls (start=False, stop=True)
are the only PE work on the recurrence cycle.

S state [128, 256] per stream (only rows 64:128 streamed, K=64): 64:80 h0,
80:96 h1, 96:112 h2, 112 bias-one (the pad feature engineered to stay exactly
1.0 through the h write: i-bias -20, f/o-bias +20, cell init 10), 113:128
zero-weighted garbage. Engine ops may only span 64 partitions when starting at
0 or 64, which fixes the whole row layout. PSUM accumulation groups must stay
consecutive per region (interleaved start/stop groups corrupt results on HW).
"""

import os
import sys

sys.path.insert(0, "/opt/trn_rl_repo")

import numpy as np

import concourse.bacc as bacc
import concourse.mybir as mybir
from concourse.tile import TileContext
from concourse import bass_utils

B, T, IN, H, L = 4096, 1024, 2, 16, 3
NCORES = 8
BL = B // NCORES        # 512 per core
SB = BL // 2            # 256 per stream
F32 = mybir.dt.float32
F16 = mybir.dt.float16
NPF16 = np.float16

# PyTorch gate rows in W_ih*/W_hh*: i, f, g, o
PT = {"i": slice(0, 16), "f": slice(16, 32), "g": slice(32, 48), "o": slice(48, 64)}

_STEPS_ENV = int(os.environ.get("LSTM_STEPS", "0"))
_CDT_ENV = os.environ.get("LSTM_CDT", "f16")  # cell-state dtype: f32 | f16
_POOL_H_ENV = os.environ.get("LSTM_POOL_H", "0") == "1"  # h-mul on gpsimd


def _t_run():
    return _STEPS_ENV if _STEPS_ENV > 0 else T


def build_weight_block(Ws):
    """WT [96, 256] f32: cols 0:128 {i|f}, 128:256 {g|o}; within a block cols
    0:64 first gate, 64:128 second; feature = 16*l + j.
    Rows (only 64:128 streamed, K=64): 64:80 h0, 80:96 h1, 96:112 h2,
    112 bias. Row 112 of S is the pad feature 48, engineered to stay exactly
    1.0: its i-gate bias is -20 (i=0), f/o-gate biases +20 (f=o=1), and its
    cell is initialized to 10, so h_pad = 1*tanh(10) == 1.0 in fp16."""
    WT = np.zeros((128, 256), np.float32)
    for bank, gates in ((0, ("i", "f")), (1, ("g", "o"))):
        for gi, gate in enumerate(gates):
            sc = 2.0 if gate == "g" else 1.0
            pt = PT[gate]
            for l in range(3):
                Wih, Whh, b = Ws[l]
                c0 = bank * 128 + 64 * gi + 16 * l
                if l > 0:
                    WT[64 + 16 * (l - 1):80 + 16 * (l - 1), c0:c0 + 16] = \
                        sc * Wih[pt].T
                WT[64 + 16 * l:80 + 16 * l, c0:c0 + 16] = sc * Whh[pt].T
                WT[112, c0:c0 + 16] = sc * b[pt]
            pc = bank * 128 + 64 * gi + 48      # pad feature col
            WT[112, pc] = {"i": -20.0, "f": 20.0, "o": 20.0, "g": 0.0}[gate]
    return WT


def build_x_weights(W_ih0):
    """wx [32, 16*256] f32: variant v=t%16 at cols v*256; within: 0:128
    {i|f}-block, 128:256 {g|o}-block; only rows 2v:2v+2 nonzero (L0 features,
    cols +0:16 of each 64-gate half)."""
    wx = np.zeros((32, 16 * 256), np.float32)
    for v in range(16):
        for bank, gates in ((0, ("i", "f")), (1, ("g", "o"))):
            for gi, gate in enumerate(gates):
                sc = 2.0 if gate == "g" else 1.0
                c0 = v * 256 + bank * 128 + 64 * gi
                wx[2 * v:2 * v + 2, c0:c0 + 16] = sc * W_ih0[PT[gate]].T
    return wx


def build_bass():
    CDT = F32 if _CDT_ENV == "f32" else F16
    NPC = np.float32 if _CDT_ENV == "f32" else NPF16
    nc = bacc.Bacc("TRN2", target_bir_lowering=False, debug=False,
                   num_devices=NCORES)
    tr = _t_run()
    NT = tr + 2
    nblk16 = (tr + 15) // 16

    xt_d = nc.dram_tensor("xt", [32, nblk16 * BL], F16, kind="ExternalInput")
    wt_d = nc.dram_tensor("wt", [128, 256], F16, kind="ExternalInput")
    wx_d = nc.dram_tensor("wx", [32, 16 * 256], F16, kind="ExternalInput")
    wfc_d = nc.dram_tensor("wfc", [17, 1], F16, kind="ExternalInput")
    s0_d = [nc.dram_tensor(f"s0{c}", [128, SB], F16, kind="ExternalInput")
            for c in "ab"]
    c0_d = [nc.dram_tensor(f"c0{c}", [64, SB], mybir.dt.from_np(np.dtype(NPC)),
                           kind="ExternalInput") for c in "ab"]
    # delayed re-inits (overwrite wavefront startup pollution)
    ri_d = {}
    for c in "ab":
        for nm in ("h1i", "h2i"):
            ri_d[nm + c] = nc.dram_tensor(nm + c, [16, SB], F16,
                                          kind="ExternalInput")
        for nm in ("c1i", "c2i"):
            ri_d[nm + c] = nc.dram_tensor(
                nm + c, [16, SB], mybir.dt.from_np(np.dtype(NPC)),
                kind="ExternalInput")
    y_d = nc.dram_tensor("y", [1, BL], F32, kind="ExternalOutput")

    SIG = mybir.ActivationFunctionType.Sigmoid
    TANH = mybir.ActivationFunctionType.Tanh

    with TileContext(nc) as tc:
        xt = nc.alloc_sbuf_tensor("xt_sb", [32, nblk16 * BL], F16)
        wt = nc.alloc_sbuf_tensor("wt_sb", [128, 256], F16)
        wx = nc.alloc_sbuf_tensor("wx_sb", [32, 16 * 256], F16)
        wfc = nc.alloc_sbuf_tensor("wfc_sb", [17, 1], F16)
        St = [nc.alloc_sbuf_tensor(f"St{c}", [128, SB], F16) for c in "ab"]
        C = [nc.alloc_sbuf_tensor(f"C{c}", [128, SB], CDT) for c in "ab"]
        hf = nc.alloc_sbuf_tensor("hf_sb", [17, BL], F16)
        ys = nc.alloc_sbuf_tensor("ys_sb", [1, BL], F32)

        nc.sync.dma_start(xt[:, :], xt_d.ap())
        nc.sync.dma_start(wt[:, :], wt_d.ap())
        nc.sync.dma_start(wx[:, :], wx_d.ap())
        nc.sync.dma_start(wfc[:, :], wfc_d.ap())
        for k in range(2):
            nc.sync.dma_start(St[k][:, :], s0_d[k].ap())
            nc.sync.dma_start(C[k][64:128, :], c0_d[k].ap())

        with tc.tile_pool(name="psGoA", bufs=2, space="PSUM") as pgA, \
             tc.tile_pool(name="psGoB", bufs=2, space="PSUM") as pgB, \
             tc.tile_pool(name="psIfA", bufs=2, space="PSUM") as piA, \
             tc.tile_pool(name="psIfB", bufs=2, space="PSUM") as piB, \
             tc.tile_pool(name="sbA", bufs=4) as psbA, \
             tc.tile_pool(name="sbB", bufs=4) as psbB:
            pgo = [pgA, pgB]
            pif = [piA, piB]
            psb = [psbA, psbB]

            # x gate contributions are prefilled into the NEXT step's PSUM
            # tiles (start=True) at the end of each stream block, so only the
            # h-dependent matmuls (start=False) sit on the recurrence cycle.
            # The go/if halves live in SEPARATE pools (separate banks): within
            # any one bank, accumulation groups stay strictly sequential --
            # interleaving two open groups in one bank corrupts results.
            def prefill_x(k, s):
                v = (s % 16) * 256
                xf0 = (s // 16) * BL
                Pg = pgo[k].tile([128, 256], F32, tag="Pg")
                Pi = pif[k].tile([128, 256], F32, tag="Pi")
                xm = xt[0:32, xf0 + k * SB:xf0 + k * SB + SB]
                nc.tensor.matmul(Pg[0:128, 0:256],
                                 wx[0:32, v + 128:v + 256], xm,
                                 start=True, stop=False)
                nc.tensor.matmul(Pi[0:128, 0:256], wx[0:32, v:v + 128], xm,
                                 start=True, stop=False)
                return Pg, Pi

            Ptil = [prefill_x(k, 0) for k in range(2)]
            for s in range(NT):
                have_x = s < tr
                for k in range(2):
                    if have_x:
                        Pg, Pi = Ptil[k]
                    else:
                        Pg = pgo[k].tile([128, 256], F32, tag="Pg")
                        Pi = pif[k].tile([128, 256], F32, tag="Pi")
                    G = psb[k].tile([128, 512], F16, tag="G")
                    TG = psb[k].tile([64, 256], F16, tag="TG")
                    U = psb[k].tile([64, 256], F16, tag="U")
                    FW = psb[k].tile([64, 256], CDT, tag="FW")
                    TC = psb[k].tile([128, 256], F16, tag="TC")

                    nc.tensor.matmul(Pg[0:128, 0:256], wt[64:128, 128:256],
                                     St[k][64:128, :], start=not have_x,
                                     stop=True)
                    nc.tensor.matmul(Pi[0:128, 0:256], wt[64:128, 0:128],
                                     St[k][64:128, :], start=not have_x,
                                     stop=True)
                    # sigmoid per half ({g|o} first: it unblocks the DVE chain)
                    nc.scalar.activation(G[0:128, 256:512], Pg[0:128, 0:256],
                                         SIG)
                    nc.scalar.activation(G[0:128, 0:256], Pi[0:128, 0:256],
                                         SIG)
                    # tg = tanh(g) = 2*sigmoid(2g) - 1
                    nc.vector.tensor_scalar(
                        TG[0:64, 0:256], G[0:64, 256:512], 2.0, -1.0,
                        mybir.AluOpType.mult, mybir.AluOpType.add)
                    # fw = f * c
                    nc.vector.tensor_mul(FW[0:64, 0:256], G[64:128, 0:256],
                                         C[k][64:128, :])
                    # u = i * tg
                    nc.vector.tensor_mul(U[0:64, 0:256], TG[0:64, 0:256],
                                         G[0:64, 0:256])
                    # c' = fw + u
                    nc.vector.tensor_add(C[k][64:128, :], FW[0:64, 0:256],
                                         U[0:64, 0:256])
                    # tc = tanh(c')
                    nc.scalar.activation(TC[64:128, 0:256], C[k][64:128, :],
                                         TANH)
                    # h = o * tc -> all three layers' h rows at once
                    eng = nc.gpsimd if _POOL_H_ENV else nc.vector
                    eng.tensor_mul(St[k][64:128, :], G[64:128, 256:512],
                                   TC[64:128, 0:256])
                    if s + 1 < tr:
                        Ptil[k] = prefill_x(k, s + 1)

                # delayed init: overwrite wavefront-startup pollution
                if s == 0:
                    for k, c in enumerate("ab"):
                        nc.sync.dma_start(St[k][80:96, :], ri_d["h1i" + c].ap())
                        nc.sync.dma_start(C[k][80:96, :], ri_d["c1i" + c].ap())
                elif s == 1:
                    for k, c in enumerate("ab"):
                        nc.sync.dma_start(St[k][96:112, :], ri_d["h2i" + c].ap())
                        nc.sync.dma_start(C[k][96:112, :], ri_d["c2i" + c].ap())

            # final fc: y = h2 @ W_fc.T + b_fc
            nc.vector.memset(hf[0:17, :], 1.0)
            for k in range(2):
                nc.vector.tensor_copy(hf[0:16, k * SB:(k + 1) * SB],
                                      St[k][96:112, :])
            for k in range(2):
                Pf = pgo[k].tile([128, 256], F32, tag="Pg")
                nc.tensor.matmul(Pf[0:1, 0:256], wfc[0:17, 0:1],
                                 hf[0:17, k * SB:(k + 1) * SB],
                                 start=True, stop=True)
                nc.scalar.copy(ys[0:1, k * SB:(k + 1) * SB], Pf[0:1, 0:256])
            nc.sync.dma_start(y_d.ap(), ys[0:1, :])

    nc.compile()
    return nc


def prep_core_inputs(inputs, core):
    NPC = np.float32 if _CDT_ENV == "f32" else NPF16
    b0, b1 = core * BL, (core + 1) * BL
    tr = _t_run()
    nblk16 = (tr + 15) // 16

    x = np.asarray(inputs["x"])[b0:b1]          # [BL, T, IN]
    h0 = np.asarray(inputs["h0"])[:, b0:b1]     # [L, BL, H]
    c0 = np.asarray(inputs["c0"])[:, b0:b1]

    # xt layout: partition = 2*(t%16)+f, free = (t//16)*BL + b
    xt = np.zeros((32, nblk16 * BL), np.float32)
    xr = x[:, :tr, :].transpose(1, 2, 0)         # [t, f, b]
    for tb in range(nblk16):
        t1 = min(tb * 16 + 16, tr)
        chunk = xr[tb * 16:t1]                   # [u, f, b]
        xt[:2 * (t1 - tb * 16), tb * BL:(tb + 1) * BL] = chunk.reshape(-1, BL)

    m = {"xt": xt.astype(NPF16)}
    for k, c in enumerate("ab"):
        sl = slice(k * SB, (k + 1) * SB)
        s0 = np.zeros((128, SB), np.float32)
        s0[112] = 1.0
        for l in range(3):
            s0[64 + 16 * l:80 + 16 * l] = h0[l, sl].T
        c0b = np.zeros((64, SB), np.float32)
        c0b[48] = 10.0
        for l in range(3):
            c0b[16 * l:16 * l + 16] = c0[l, sl].T
        m["s0" + c] = s0.astype(NPF16)
        m["c0" + c] = c0b.astype(NPC)
        m["h1i" + c] = np.ascontiguousarray(h0[1, sl].T).astype(NPF16)
        m["c1i" + c] = np.ascontiguousarray(c0[1, sl].T).astype(NPC)
        m["h2i" + c] = np.ascontiguousarray(h0[2, sl].T).astype(NPF16)
        m["c2i" + c] = np.ascontiguousarray(c0[2, sl].T).astype(NPC)
    return m


_NC_CACHE = {}


def kernel(**inputs):
    key = (_t_run(), _CDT_ENV, _POOL_H_ENV)
    if key not in _NC_CACHE:
        _NC_CACHE[key] = build_bass()
    nc = _NC_CACHE[key]

    Ws = []
    for l in range(3):
        Ws.append((np.asarray(inputs[f"W_ih{l}"]), np.asarray(inputs[f"W_hh{l}"]),
                   np.asarray(inputs[f"b_ih{l}"]) + np.asarray(inputs[f"b_hh{l}"])))
    WT = build_weight_block(Ws).astype(NPF16)
    WX = build_x_weights(Ws[0][0]).astype(NPF16)
    wfc = np.zeros((17, 1), np.float32)
    wfc[0:16, 0] = np.asarray(inputs["W_fc"])[0]
    wfc[16, 0] = np.asarray(inputs["b_fc"])[0]
    wfc = wfc.astype(NPF16)

    in_maps = []
    for core in range(NCORES):
        m = prep_core_inputs(inputs, core)
        m["wt"] = WT
        m["wx"] = WX
        m["wfc"] = wfc
        in_maps.append(m)

    trace = os.environ.get("LSTM_TRACE", "0") == "1"
    tmpdir = os.environ.get("LSTM_TMPDIR") or None
    res = bass_utils.run_bass_kernel_spmd(nc, in_maps, core_ids=list(range(NCORES)),
                                          trace=trace, tmpdir=tmpdir)
    global _LAST_RESULT
    _LAST_RESULT = res
    out = np.concatenate([res.results[c]["y"][0] for c in range(NCORES)])
    return out.reshape(B, 1).astype(np.float32)


_LAST_RESULT = None


if __name__ == "__main__":
    import reference
    inputs = reference.setup_inputs()
    y = kernel(**{k: np.asarray(v) for k, v in inputs.items()})
    print("kernel out", y.shape, y[:4, 0])
